# revision 2
# baseline (speedup 1.0000x reference)
"""3-layer GAT (PyG GATConv x3 + global mean pool) on 8 trn2 NeuronCores, v2.

Single fused SPMD program (all 3 layers).  Nodes dealt round-robin by
descending in-degree to 8 cores (dst-sharding).  Per layer:
 - dense phase in TRANSPOSED layout: psum_hT = W^T @ hT_in (PE), attention
   logit halves via a second small matmul, PE transposes back to node-major
   rows [h+b | al_s] staged to a DRAM table,
 - one AllGather replicates the table,
 - edge phase: per block of dst chunks, ONE multi-index indirect DMA gathers
   all slot rows, DVE computes es = exp(lrelu(al_s+al_d)+B), denominator via
   tensor_reduce, weighted message fold-tree, normalize (+relu), PE-transpose
   into the next layer's hT.
Layer 3 accumulates a column sum on PE; host divides by N and adds b3.
"""
import numpy as np
import concourse.bass as bass
import concourse.bacc as bacc
import concourse.mybir as mybir
import concourse.tile as tile
from concourse.masks import make_identity

P = 128
NEG_SLOPE = 0.2
PAD_ALS = -30000.0
EXP_BIAS = -3.0
F32 = mybir.dt.float32
F16 = mybir.dt.float16
I32 = mybir.dt.int32
BATCH_SLOTS = 96
DB = 4

N_NODES, N_EDGES = 100000, 1600000
H = (8, 8, 1)
CH = (16, 16, 32)
OC = tuple(H[i] * CH[i] for i in range(3))
RL = (136, 136, 34)


class Plan:
    pass


def make_plan(edge_index, N, ncores=8, kstep=1):
    E = edge_index.shape[1]
    src = np.concatenate([edge_index[0].astype(np.int64), np.arange(N, dtype=np.int64)])
    dst = np.concatenate([edge_index[1].astype(np.int64), np.arange(N, dtype=np.int64)])
    deg = np.bincount(dst, minlength=N)
    order = np.argsort(-deg, kind="stable")

    npc = (N + ncores - 1) // ncores
    n_chunks = (npc + P - 1) // P + 1  # last chunk is all-pad
    S = n_chunks * P

    Ks = []
    for j in range(n_chunks):
        g0 = j * P * ncores
        dmax = int(deg[order[g0]]) if g0 < N else 1
        Ks.append(max(kstep, ((dmax + kstep - 1) // kstep) * kstep))
    Ks = np.array(Ks, np.int32)
    TK = int(Ks.sum())

    node_at = np.full((ncores, S), -1, np.int64)
    for c in range(ncores):
        g = np.arange(npc) * ncores + c
        valid = g < N
        node_at[c, :npc][valid] = order[g[valid]]
    row_of = np.zeros(N, np.int64)
    for c in range(ncores):
        m = node_at[c] >= 0
        row_of[node_at[c][m]] = c * S + np.nonzero(m)[0]

    eo = np.argsort(dst, kind="stable")
    src_sorted = src[eo]
    starts = np.zeros(N + 1, np.int64)
    np.cumsum(deg, out=starts[1:])

    idx = np.zeros((ncores, P, TK), np.int32)
    maskD = np.zeros((ncores, P, n_chunks), np.float32)
    off = 0
    for j in range(n_chunks):
        K = int(Ks[j])
        for c in range(ncores):
            block = np.full((P, K), c * S + (S - 1), np.int32)
            nodes = node_at[c, j * P:(j + 1) * P]
            for p in range(P):
                n = nodes[p]
                if n < 0:
                    maskD[c, p, j] = 1.0
                    continue
                s0, s1 = int(starts[n]), int(starts[n + 1])
                block[p, :s1 - s0] = row_of[src_sorted[s0:s1]]
            idx[c, :, off:off + K] = block
        off += K

    iters = []
    j = 0
    off = 0
    while j < n_chunks:
        K = int(Ks[j])
        B = max(1, BATCH_SLOTS // K)
        nb = 1
        while nb < B and j + nb < n_chunks and Ks[j + nb] == K:
            nb += 1
        iters.append((K, j, nb, off))
        off += K * nb
        j += nb

    pl = Plan()
    pl.N, pl.E, pl.ncores = N, E, ncores
    pl.npc, pl.n_chunks, pl.S, pl.TK = npc, n_chunks, S, TK
    pl.Ks, pl.iters, pl.node_at, pl.row_of = Ks, iters, node_at, row_of
    pl.idx, pl.maskD = idx, maskD
    pl.NSMAX = max(k * nb for (k, _, nb, _) in iters)
    pl.MAXB = max(nb for (_, _, nb, _) in iters)
    pl.SC1 = max(nb * ((k + 1) // 2) for (k, _, nb, _) in iters)
    pl.SC2 = max(nb * ((((k + 1) // 2) + 1) // 2) for (k, _, nb, _) in iters)
    return pl


def x_slices(pl, x):
    out = []
    for c in range(pl.ncores):
        xs = np.zeros((pl.S, x.shape[1]), np.float16)
        m = pl.node_at[c] >= 0
        xs[m] = x[pl.node_at[c][m]].astype(np.float16)
        out.append(np.ascontiguousarray(xs.T))
    return out


def pack_av(a_src, a_dst, oc):
    """[oc, 2*heads] fp16: col j = a_src head j laid in its ch block; col heads+j = a_dst."""
    a_src = np.asarray(a_src, np.float32)
    a_dst = np.asarray(a_dst, np.float32)
    heads, ch = a_src.shape
    m = np.zeros((oc, 2 * heads), np.float32)
    for h in range(heads):
        m[h * ch:(h + 1) * ch, h] = a_src[h]
        m[h * ch:(h + 1) * ch, heads + h] = a_dst[h]
    return m.astype(np.float16)


def build_program(pl, reps=1, ncores=8):
    S, n_chunks, TK = pl.S, pl.n_chunks, pl.TK
    NC = ncores
    NSMAX, MAXB = pl.NSMAX, pl.MAXB

    nc = bacc.Bacc("TRN2", target_bir_lowering=False, debug=False, num_devices=NC)
    t_hin = nc.dram_tensor("hin", [P, S], F16, kind="ExternalInput")
    t_idx = nc.dram_tensor("idx", [P, TK], I32, kind="ExternalInput")
    t_maskD = nc.dram_tensor("maskD", [P, n_chunks], F32, kind="ExternalInput")
    t_W = [nc.dram_tensor(f"W{l+1}", [128 if l < 2 else OC[1], OC[l]], F16,
                          kind="ExternalInput") for l in range(3)]
    t_av = [nc.dram_tensor(f"av{l+1}", [OC[l], 2 * H[l]], F16,
                           kind="ExternalInput") for l in range(3)]
    t_bv = [nc.dram_tensor(f"bv{l+1}", [P, OC[l]], F16, kind="ExternalInput")
            for l in range(2)]
    t_y = nc.dram_tensor("y", [1, OC[2]], F32, kind="ExternalOutput")

    with tile.TileContext(nc) as tc:
        with tc.tile_pool(name="res", bufs=1) as res, \
             tc.tile_pool(name="dram", bufs=1, space="DRAM") as dram, \
             tc.tile_pool(name="dn", bufs=2) as dn, \
             tc.tile_pool(name="dnp", bufs=1, space="PSUM") as dnp, \
             tc.tile_pool(name="dnt", bufs=2, space="PSUM") as dnt, \
             tc.tile_pool(name="eg", bufs=2) as eg, \
             tc.tile_pool(name="eg1", bufs=1) as eg1, \
             tc.tile_pool(name="egp", bufs=2, space="PSUM") as egp, \
             tc.tile_pool(name="egy", bufs=1, space="PSUM") as egy:

            hT = [res.tile([P, S], F16, name=f"hT{i}") for i in range(2)]
            nc.sync.dma_start(hT[0][:], t_hin.ap())
            idx_sb = res.tile([P, TK], I32)
            nc.sync.dma_start(idx_sb[:], t_idx.ap())
            mask_sb = res.tile([P, n_chunks], F32)
            nc.sync.dma_start(mask_sb[:], t_maskD.ap())
            ident = res.tile([P, P], F16)
            make_identity(nc, ident[:])
            ones_col = res.tile([P, 1], F16)
            nc.gpsimd.memset(ones_col[:], 1.0)
            ebias_col = res.tile([P, 1], F32)
            nc.gpsimd.memset(ebias_col[:], EXP_BIAS)
            W_sb, av_sb, bv_sb = [], [], []
            for l in range(3):
                w = res.tile(list(t_W[l].shape), F16, name=f"Wsb{l}")
                nc.sync.dma_start(w[:], t_W[l].ap())
                W_sb.append(w)
                a = res.tile(list(t_av[l].shape), F16, name=f"avsb{l}")
                nc.sync.dma_start(a[:], t_av[l].ap())
                av_sb.append(a)
                if l < 2:
                    b = res.tile([P, OC[l]], F16, name=f"bvsb{l}")
                    nc.sync.dma_start(b[:], t_bv[l].ap())
                    bv_sb.append(b)
            alD = [res.tile([P, n_chunks * 8], F16, name=f"alD{i}") for i in range(2)]

            ysb = res.tile([1, OC[2]], F32)

            for rep in range(reps):
                hcl = {l: dram.tile([S, RL[l]], F16, name=f"hcl{l}_{rep}")
                       for l in range(3)}
                hcf = {l: dram.tile([NC * S, RL[l]], F16, name=f"hcf{l}_{rep}",
                                    addr_space="Shared") for l in range(3)}
                for l in range(3):
                    oc, heads, ch, rl = OC[l], H[l], CH[l], RL[l]
                    cin = 128 if l < 2 else OC[1]
                    hin_t = hT[l % 2]
                    hout_t = hT[(l + 1) % 2]
                    alD_l = alD[l % 2]
                    # ---------------- dense (transposed) ----------------
                    for it0 in range(0, n_chunks, DB):
                        nb = min(DB, n_chunks - it0)
                        ph = dnp.tile([oc, DB * P], F32, tag="ph")
                        for q in range(nb):
                            nc.tensor.matmul(
                                ph[:oc, q * P:(q + 1) * P], lhsT=W_sb[l][:],
                                rhs=hin_t[:cin, (it0 + q) * P:(it0 + q + 1) * P],
                                start=True, stop=True)
                        hsb = dn.tile([oc, DB * P], F16, tag="hsb")
                        nc.vector.tensor_copy(out=hsb[:oc, :nb * P],
                                              in_=ph[:oc, :nb * P])
                        pal = dnp.tile([2 * heads, DB * P], F32, tag="pal")
                        nc.tensor.matmul(pal[:2 * heads, :nb * P],
                                         lhsT=av_sb[l][:],
                                         rhs=hsb[:oc, :nb * P],
                                         start=True, stop=True)
                        alsb = dn.tile([2 * heads, DB * P], F16, tag="alsb")
                        nc.vector.tensor_copy(out=alsb[:2 * heads, :nb * P],
                                              in_=pal[:2 * heads, :nb * P])
                        hc = dn.tile([P, DB * rl], F16, tag="hc")
                        for q in range(nb):
                            tpt = dnt.tile([P, 144], F16, tag="tpt")
                            nc.tensor.transpose(
                                out=tpt[:, 0:oc],
                                in_=hsb[:oc, q * P:(q + 1) * P],
                                identity=ident[:oc, :oc])
                            nc.tensor.transpose(
                                out=tpt[:, 128:128 + 2 * heads],
                                in_=alsb[:2 * heads, q * P:(q + 1) * P],
                                identity=ident[:2 * heads, :2 * heads])
                            if l < 2:
                                nc.vector.tensor_tensor(
                                    out=hc[:, q * rl:q * rl + oc],
                                    in0=tpt[:, 0:oc], in1=bv_sb[l][:],
                                    op=mybir.AluOpType.add)
                                nc.vector.tensor_copy(
                                    out=hc[:, q * rl + oc:q * rl + oc + heads],
                                    in_=tpt[:, 128:128 + heads])
                            else:
                                nc.vector.tensor_copy(
                                    out=hc[:, q * rl:q * rl + oc],
                                    in_=tpt[:, 0:oc])
                                nc.vector.tensor_copy(
                                    out=hc[:, q * rl + oc:q * rl + oc + 2],
                                    in_=tpt[:, 128:130])
                            nc.vector.tensor_copy(
                                out=alD_l[:, (it0 + q) * 8:(it0 + q) * 8 + heads],
                                in_=tpt[:, 128 + heads:128 + 2 * heads])
                        if it0 + nb == n_chunks:
                            nc.gpsimd.memset(
                                hc[:, (nb - 1) * rl + oc:(nb - 1) * rl + oc + heads],
                                PAD_ALS)
                        nc.sync.dma_start(
                            hcl[l][:][it0 * P:(it0 + nb) * P, :]
                            .rearrange("(q p) r -> p q r", p=P),
                            hc[:, :nb * rl].rearrange("p (q r) -> p q r", r=rl))
                    # ---------------- allgather ----------------
                    nc.gpsimd.collective_compute(
                        "AllGather", mybir.AluOpType.bypass,
                        replica_groups=[list(range(NC))],
                        ins=[hcl[l].opt()], outs=[hcf[l].opt()])
                    # ---------------- edge phase ----------------
                    if l == 2:
                        ysum_ps = egy.tile([1, OC[2]], F32, tag="ysum")
                        n_mm = sum(nb for (_, _, nb, _) in pl.iters)
                        i_mm = 0
                    for (K, c0, nb, coff) in pl.iters:
                        ns = K * nb
                        g = eg.tile([P, NSMAX * RL[0]], F16, tag="g")
                        for k in range(ns):
                            nc.gpsimd.indirect_dma_start(
                                out=g[:, k * rl:(k + 1) * rl], out_offset=None,
                                in_=hcf[l][:],
                                in_offset=bass.IndirectOffsetOnAxis(
                                    ap=idx_sb[:, coff + k:coff + k + 1], axis=0))
                        gv = g[:, :ns * rl].rearrange("p (s r) -> p s r", r=rl)
                        # logits
                        lg = eg1.tile([P, NSMAX * 8], F16, tag="lg")
                        al_d_bc = alD_l[:, c0 * 8:(c0 + nb) * 8] \
                            .rearrange("p (q e) -> p q e", e=8)[:, :, :heads] \
                            .unsqueeze(2).to_broadcast([P, nb, K, heads])
                        nc.vector.tensor_tensor(
                            out=lg[:, :ns * heads].rearrange(
                                "p (q k h) -> p q k h", k=K, h=heads),
                            in0=gv[:, :, oc:oc + heads]
                            .rearrange("p (q k) h -> p q k h", k=K),
                            in1=al_d_bc, op=mybir.AluOpType.add)
                        lg2 = eg1.tile([P, NSMAX * 8], F16, tag="lg2")
                        nc.vector.tensor_scalar_mul(
                            lg2[:, :ns * heads], lg[:, :ns * heads], NEG_SLOPE)
                        lg3 = eg1.tile([P, NSMAX * 8], F16, tag="lg3")
                        nc.vector.tensor_tensor(
                            out=lg3[:, :ns * heads], in0=lg[:, :ns * heads],
                            in1=lg2[:, :ns * heads], op=mybir.AluOpType.max)
                        es = eg1.tile([P, NSMAX * 8], F16, tag="es")
                        nc.scalar.activation(
                            out=es[:, :ns * heads], in_=lg3[:, :ns * heads],
                            func=mybir.ActivationFunctionType.Exp,
                            bias=ebias_col[:])
                        # denominator
                        den = eg1.tile([P, MAXB * 8], F32, tag="den")
                        nc.vector.tensor_reduce(
                            out=den[:, :nb * heads],
                            in_=es[:, :ns * heads].rearrange(
                                "p (q k h) -> p q h k", k=K, h=heads),
                            axis=mybir.AxisListType.X, op=mybir.AluOpType.add)
                        den2 = eg1.tile([P, MAXB * 8], F32, tag="den2")
                        m_bc = mask_sb[:, c0:c0 + nb].unsqueeze(2) \
                            .to_broadcast([P, nb, heads])
                        nc.vector.tensor_tensor(
                            out=den2[:, :nb * heads].rearrange(
                                "p (q h) -> p q h", h=heads),
                            in0=den[:, :nb * heads].rearrange(
                                "p (q h) -> p q h", h=heads),
                            in1=m_bc, op=mybir.AluOpType.add)
                        inv = eg1.tile([P, MAXB * 8], F32, tag="inv")
                        nc.vector.reciprocal(out=inv[:, :nb * heads],
                                             in_=den2[:, :nb * heads])
                        # weighted messages
                        mp = eg1.tile([P, NSMAX * 128], F16, tag="mp")
                        es_bc = es[:, :ns * heads].rearrange(
                            "p (s h) -> p s h", h=heads) \
                            .unsqueeze(3).to_broadcast([P, ns, heads, ch])
                        nc.vector.tensor_tensor(
                            out=mp[:, :ns * oc].rearrange(
                                "p (s h c) -> p s h c", h=heads, c=ch),
                            in0=gv[:, :, 0:oc].rearrange(
                                "p s (h c) -> p s h c", h=heads),
                            in1=es_bc, op=mybir.AluOpType.mult)
                        # fold-tree reduce over K
                        scrA = eg1.tile([P, pl.SC1 * 128], F16, tag="scrA")
                        scrB = eg1.tile([P, pl.SC2 * 128], F16, tag="scrB")
                        cur, curk = mp, K
                        while curk > 1:
                            a_in = cur[:, :nb * curk * oc].rearrange(
                                "p (q k c) -> p q k c", k=curk, c=oc)
                            half = (curk + 1) // 2
                            pair = curk - half
                            dst_t = scrA if cur is not scrA else scrB
                            o_v = dst_t[:, :nb * half * oc].rearrange(
                                "p (q k c) -> p q k c", k=half, c=oc)
                            nc.vector.tensor_tensor(
                                out=o_v[:, :, 0:pair], in0=a_in[:, :, 0:pair],
                                in1=a_in[:, :, half:half + pair],
                                op=mybir.AluOpType.add)
                            if half > pair:
                                nc.vector.tensor_copy(
                                    out=o_v[:, :, pair:half],
                                    in_=a_in[:, :, pair:half])
                            cur, curk = dst_t, half
                        # normalize
                        hout = eg.tile([P, MAXB * 128], F16, tag="hout")
                        inv_bc = inv[:, :nb * heads].rearrange(
                            "p (q h) -> p q h", h=heads) \
                            .unsqueeze(3).to_broadcast([P, nb, heads, ch])
                        nc.vector.tensor_tensor(
                            out=hout[:, :nb * oc].rearrange(
                                "p (q h c) -> p q h c", h=heads, c=ch),
                            in0=cur[:, :nb * oc].rearrange(
                                "p (q h c) -> p q h c", h=heads, c=ch),
                            in1=inv_bc, op=mybir.AluOpType.mult)
                        if l < 2:
                            hr = eg.tile([P, MAXB * 128], F16, tag="hr")
                            nc.scalar.activation(
                                out=hr[:, :nb * oc], in_=hout[:, :nb * oc],
                                func=mybir.ActivationFunctionType.Relu)
                            for q in range(nb):
                                tp2 = egp.tile([P, P], F16, tag="tp2")
                                nc.tensor.transpose(
                                    out=tp2[:], in_=hr[:, q * oc:(q + 1) * oc],
                                    identity=ident[:])
                                nc.vector.tensor_copy(
                                    out=hout_t[:, (c0 + q) * P:(c0 + q + 1) * P],
                                    in_=tp2[:])
                        else:
                            for q in range(nb):
                                nc.tensor.matmul(
                                    ysum_ps[:], lhsT=ones_col[:],
                                    rhs=hout[:, q * oc:(q + 1) * oc],
                                    start=(i_mm == 0), stop=(i_mm == n_mm - 1))
                                i_mm += 1
                nc.vector.tensor_copy(out=ysb[:], in_=ysum_ps[:])
            nc.sync.dma_start(t_y.ap(), ysb[:])
    nc.compile()
    return nc


# ----------------------------------------------------------------- entry point

_CACHE = {}


def _get_compiled(edge_index, reps=1):
    key = (hash(edge_index.tobytes()), reps)
    if key not in _CACHE:
        pl = _CACHE.get(("plan", hash(edge_index.tobytes())))
        if pl is None:
            pl = make_plan(edge_index, N_NODES, ncores=8)
            _CACHE[("plan", hash(edge_index.tobytes()))] = pl
        _CACHE[key] = (pl, build_program(pl, reps=reps))
    return _CACHE[key]


def make_inputs(pl, x, edge_index, W1, a_src1, a_dst1, b1, W2, a_src2, a_dst2,
                b2, W3, a_src3, a_dst3, b3):
    hins = x_slices(pl, np.asarray(x, np.float32))
    Ws = [np.asarray(W, np.float16) for W in (W1, W2, W3)]
    avs = [pack_av(a_src1, a_dst1, OC[0]), pack_av(a_src2, a_dst2, OC[1]),
           pack_av(a_src3, a_dst3, OC[2])]
    bvs = [np.tile(np.asarray(b, np.float16)[None, :], (P, 1)) for b in (b1, b2)]
    in_maps = []
    for c in range(pl.ncores):
        in_maps.append({
            "hin": hins[c], "idx": pl.idx[c], "maskD": pl.maskD[c],
            "W1": Ws[0], "W2": Ws[1], "W3": Ws[2],
            "av1": avs[0], "av2": avs[1], "av3": avs[2],
            "bv1": bvs[0], "bv2": bvs[1],
        })
    return in_maps


def kernel(x, edge_index, W1, a_src1, a_dst1, b1, W2, a_src2, a_dst2, b2,
           W3, a_src3, a_dst3, b3):
    from concourse import bass_utils
    edge_index = np.asarray(edge_index, np.int32)
    pl, nc = _get_compiled(edge_index)
    in_maps = make_inputs(pl, x, edge_index, W1, a_src1, a_dst1, b1,
                          W2, a_src2, a_dst2, b2, W3, a_src3, a_dst3, b3)
    res = bass_utils.run_bass_kernel_spmd(nc, in_maps, core_ids=list(range(8)))
    tot = np.sum([res.results[c]["y"] for c in range(8)], axis=0)
    return (tot / np.float32(N_NODES)
            + np.asarray(b3, np.float32)[None, :]).astype(np.float32)


# revision 4
# speedup vs baseline: 1.0988x; 1.0988x over previous
"""3-layer GAT (PyG GATConv x3 + global mean pool) on 8 trn2 NeuronCores, v2.

Single fused SPMD program (all 3 layers).  Nodes dealt round-robin by
descending in-degree to 8 cores (dst-sharding).  Per layer:
 - dense phase in TRANSPOSED layout: psum_hT = W^T @ hT_in (PE), attention
   logit halves via a second small matmul, PE transposes back to node-major
   rows [h+b | al_s] staged to a DRAM table,
 - one AllGather replicates the table,
 - edge phase: per block of dst chunks, ONE multi-index indirect DMA gathers
   all slot rows, DVE computes es = exp(lrelu(al_s+al_d)+B), denominator via
   tensor_reduce, weighted message fold-tree, normalize (+relu), PE-transpose
   into the next layer's hT.
Layer 3 accumulates a column sum on PE; host divides by N and adds b3.
"""
import numpy as np
import concourse.bass as bass
import concourse.bacc as bacc
import concourse.mybir as mybir
import concourse.tile as tile
from concourse.masks import make_identity

P = 128
NEG_SLOPE = 0.2
PAD_ALS = -30000.0
EXP_BIAS = -3.0
F32 = mybir.dt.float32
F16 = mybir.dt.float16
I32 = mybir.dt.int32
BATCH_SLOTS = 96
DB = 4

N_NODES, N_EDGES = 100000, 1600000
H = (8, 8, 1)
CH = (16, 16, 32)
OC = tuple(H[i] * CH[i] for i in range(3))
RL = (136, 136, 34)


class Plan:
    pass


def make_plan(edge_index, N, ncores=8, kstep=1):
    E = edge_index.shape[1]
    src = np.concatenate([edge_index[0].astype(np.int64), np.arange(N, dtype=np.int64)])
    dst = np.concatenate([edge_index[1].astype(np.int64), np.arange(N, dtype=np.int64)])
    deg = np.bincount(dst, minlength=N)
    order = np.argsort(-deg, kind="stable")

    npc = (N + ncores - 1) // ncores
    n_chunks = (npc + P - 1) // P + 1  # last chunk is all-pad
    S = n_chunks * P

    Ks = []
    for j in range(n_chunks):
        g0 = j * P * ncores
        dmax = int(deg[order[g0]]) if g0 < N else 1
        Ks.append(max(kstep, ((dmax + kstep - 1) // kstep) * kstep))
    Ks = np.array(Ks, np.int32)
    TK = int(Ks.sum())

    node_at = np.full((ncores, S), -1, np.int64)
    for c in range(ncores):
        g = np.arange(npc) * ncores + c
        valid = g < N
        node_at[c, :npc][valid] = order[g[valid]]
    row_of = np.zeros(N, np.int64)
    for c in range(ncores):
        m = node_at[c] >= 0
        row_of[node_at[c][m]] = c * S + np.nonzero(m)[0]

    eo = np.argsort(dst, kind="stable")
    src_sorted = src[eo]
    starts = np.zeros(N + 1, np.int64)
    np.cumsum(deg, out=starts[1:])

    idx = np.zeros((ncores, P, TK), np.int32)
    maskD = np.zeros((ncores, P, n_chunks), np.float32)
    off = 0
    for j in range(n_chunks):
        K = int(Ks[j])
        for c in range(ncores):
            block = np.full((P, K), c * S + (S - 1), np.int32)
            nodes = node_at[c, j * P:(j + 1) * P]
            for p in range(P):
                n = nodes[p]
                if n < 0:
                    maskD[c, p, j] = 1.0
                    continue
                s0, s1 = int(starts[n]), int(starts[n + 1])
                block[p, :s1 - s0] = row_of[src_sorted[s0:s1]]
            idx[c, :, off:off + K] = block
        off += K

    iters = []
    j = 0
    off = 0
    while j < n_chunks:
        K = int(Ks[j])
        B = max(1, BATCH_SLOTS // K)
        nb = 1
        while nb < B and j + nb < n_chunks and Ks[j + nb] == K:
            nb += 1
        iters.append((K, j, nb, off))
        off += K * nb
        j += nb

    pl = Plan()
    pl.N, pl.E, pl.ncores = N, E, ncores
    pl.npc, pl.n_chunks, pl.S, pl.TK = npc, n_chunks, S, TK
    pl.Ks, pl.iters, pl.node_at, pl.row_of = Ks, iters, node_at, row_of
    pl.idx, pl.maskD = idx, maskD
    pl.NSMAX = max(k * nb for (k, _, nb, _) in iters)
    pl.MAXB = max(nb for (_, _, nb, _) in iters)
    pl.SC1 = max(nb * ((k + 1) // 2) for (k, _, nb, _) in iters)
    pl.SC2 = max(nb * ((((k + 1) // 2) + 1) // 2) for (k, _, nb, _) in iters)
    return pl


def x_slices(pl, x):
    out = []
    for c in range(pl.ncores):
        xs = np.zeros((pl.S, x.shape[1]), np.float16)
        m = pl.node_at[c] >= 0
        xs[m] = x[pl.node_at[c][m]].astype(np.float16)
        out.append(np.ascontiguousarray(xs.T))
    return out


def pack_av(a_src, a_dst, oc):
    """[oc, 2*heads] fp16: col j = a_src head j laid in its ch block; col heads+j = a_dst."""
    a_src = np.asarray(a_src, np.float32)
    a_dst = np.asarray(a_dst, np.float32)
    heads, ch = a_src.shape
    m = np.zeros((oc, 2 * heads), np.float32)
    for h in range(heads):
        m[h * ch:(h + 1) * ch, h] = a_src[h]
        m[h * ch:(h + 1) * ch, heads + h] = a_dst[h]
    return m.astype(np.float16)


def build_program(pl, reps=1, ncores=8):
    S, n_chunks, TK = pl.S, pl.n_chunks, pl.TK
    NC = ncores
    NSMAX, MAXB = pl.NSMAX, pl.MAXB

    nc = bacc.Bacc("TRN2", target_bir_lowering=False, debug=False, num_devices=NC)
    t_hin = nc.dram_tensor("hin", [P, S], F16, kind="ExternalInput")
    t_idx = nc.dram_tensor("idx", [P, TK], I32, kind="ExternalInput")
    t_maskD = nc.dram_tensor("maskD", [P, n_chunks], F32, kind="ExternalInput")
    t_W = [nc.dram_tensor(f"W{l+1}", [128 if l < 2 else OC[1], OC[l]], F16,
                          kind="ExternalInput") for l in range(3)]
    t_av = [nc.dram_tensor(f"av{l+1}", [OC[l], 2 * H[l]], F16,
                           kind="ExternalInput") for l in range(3)]
    t_bv = [nc.dram_tensor(f"bv{l+1}", [P, OC[l]], F16, kind="ExternalInput")
            for l in range(2)]
    t_y = nc.dram_tensor("y", [1, OC[2]], F32, kind="ExternalOutput")

    with tile.TileContext(nc) as tc:
        with tc.tile_pool(name="res", bufs=1) as res, \
             tc.tile_pool(name="dram", bufs=1, space="DRAM") as dram, \
             tc.tile_pool(name="dn", bufs=2) as dn, \
             tc.tile_pool(name="dnp", bufs=1, space="PSUM") as dnp, \
             tc.tile_pool(name="dnt", bufs=2, space="PSUM") as dnt, \
             tc.tile_pool(name="eg", bufs=2) as eg, \
             tc.tile_pool(name="eg1", bufs=1) as eg1, \
             tc.tile_pool(name="egp", bufs=2, space="PSUM") as egp, \
             tc.tile_pool(name="egy", bufs=1, space="PSUM") as egy:

            hT = [res.tile([P, S], F16, name=f"hT{i}") for i in range(2)]
            nc.sync.dma_start(hT[0][:], t_hin.ap())
            idx_sb = res.tile([P, TK], I32)
            nc.sync.dma_start(idx_sb[:], t_idx.ap())
            mask_sb = res.tile([P, n_chunks], F32)
            nc.sync.dma_start(mask_sb[:], t_maskD.ap())
            ident = res.tile([P, P], F16)
            make_identity(nc, ident[:])
            ones_col = res.tile([P, 1], F16)
            nc.gpsimd.memset(ones_col[:], 1.0)
            ebias_col = res.tile([P, 1], F32)
            nc.gpsimd.memset(ebias_col[:], EXP_BIAS)
            W_sb, av_sb, bv_sb = [], [], []
            for l in range(3):
                w = res.tile(list(t_W[l].shape), F16, name=f"Wsb{l}")
                nc.sync.dma_start(w[:], t_W[l].ap())
                W_sb.append(w)
                a = res.tile(list(t_av[l].shape), F16, name=f"avsb{l}")
                nc.sync.dma_start(a[:], t_av[l].ap())
                av_sb.append(a)
                if l < 2:
                    b = res.tile([P, OC[l]], F16, name=f"bvsb{l}")
                    nc.sync.dma_start(b[:], t_bv[l].ap())
                    bv_sb.append(b)
            alD = [res.tile([P, n_chunks * 8], F16, name=f"alD{i}") for i in range(2)]

            ysb = res.tile([1, OC[2]], F32)

            for rep in range(reps):
                hcl = {l: dram.tile([S, RL[l]], F16, name=f"hcl{l}_{rep}")
                       for l in range(3)}
                hcf = {l: dram.tile([NC * S, RL[l]], F16, name=f"hcf{l}_{rep}",
                                    addr_space="Shared") for l in range(3)}
                def emit_dense(dl, it0):
                    """Transposed dense block for layer dl, chunks it0..it0+nb."""
                    oc, heads, rl = OC[dl], H[dl], RL[dl]
                    cin = 128 if dl < 2 else OC[1]
                    hin_t = hT[dl % 2]
                    alD_d = alD[dl % 2]
                    nb = min(DB, n_chunks - it0)
                    ph = dnp.tile([oc, DB * P], F32, tag="ph")
                    for q in range(nb):
                        nc.tensor.matmul(
                            ph[:oc, q * P:(q + 1) * P], lhsT=W_sb[dl][:],
                            rhs=hin_t[:cin, (it0 + q) * P:(it0 + q + 1) * P],
                            start=True, stop=True)
                    hsb = dn.tile([oc, DB * P], F16, tag="hsb")
                    nc.vector.tensor_copy(out=hsb[:oc, :nb * P],
                                          in_=ph[:oc, :nb * P])
                    pal = dnp.tile([2 * heads, DB * P], F32, tag="pal")
                    nc.tensor.matmul(pal[:2 * heads, :nb * P],
                                     lhsT=av_sb[dl][:],
                                     rhs=hsb[:oc, :nb * P],
                                     start=True, stop=True)
                    alsb = dn.tile([2 * heads, DB * P], F16, tag="alsb")
                    nc.vector.tensor_copy(out=alsb[:2 * heads, :nb * P],
                                          in_=pal[:2 * heads, :nb * P])
                    hc = dn.tile([P, DB * rl], F16, tag="hc")
                    for q in range(nb):
                        tpt = dnt.tile([P, 144], F16, tag="tpt")
                        nc.tensor.transpose(
                            out=tpt[:, 0:oc],
                            in_=hsb[:oc, q * P:(q + 1) * P],
                            identity=ident[:oc, :oc])
                        nc.tensor.transpose(
                            out=tpt[:, 128:128 + 2 * heads],
                            in_=alsb[:2 * heads, q * P:(q + 1) * P],
                            identity=ident[:2 * heads, :2 * heads])
                        if dl < 2:
                            nc.vector.tensor_tensor(
                                out=hc[:, q * rl:q * rl + oc],
                                in0=tpt[:, 0:oc], in1=bv_sb[dl][:],
                                op=mybir.AluOpType.add)
                            nc.vector.tensor_copy(
                                out=hc[:, q * rl + oc:q * rl + oc + heads],
                                in_=tpt[:, 128:128 + heads])
                        else:
                            nc.vector.tensor_copy(
                                out=hc[:, q * rl:q * rl + oc],
                                in_=tpt[:, 0:oc])
                            nc.vector.tensor_copy(
                                out=hc[:, q * rl + oc:q * rl + oc + 2],
                                in_=tpt[:, 128:130])
                        nc.vector.tensor_copy(
                            out=alD_d[:, (it0 + q) * 8:(it0 + q) * 8 + heads],
                            in_=tpt[:, 128 + heads:128 + 2 * heads])
                    if it0 + nb == n_chunks:
                        nc.gpsimd.memset(
                            hc[:, (nb - 1) * rl + oc:(nb - 1) * rl + oc + heads],
                            PAD_ALS)
                    nc.sync.dma_start(
                        hcl[dl][:][it0 * P:(it0 + nb) * P, :]
                        .rearrange("(q p) r -> p q r", p=P),
                        hc[:, :nb * rl].rearrange("p (q r) -> p q r", r=rl))

                # layer-0 dense up front; dense(l+1) is interleaved into
                # edge(l) below so it hides under the Pool-serial gathers
                for it0 in range(0, n_chunks, DB):
                    emit_dense(0, it0)
                for l in range(3):
                    oc, heads, ch, rl = OC[l], H[l], CH[l], RL[l]
                    hout_t = hT[(l + 1) % 2]
                    alD_l = alD[l % 2]
                    next_d = 0
                    # ---------------- allgather ----------------
                    nc.gpsimd.collective_compute(
                        "AllGather", mybir.AluOpType.bypass,
                        replica_groups=[list(range(NC))],
                        ins=[hcl[l].opt()], outs=[hcf[l].opt()])
                    # ---------------- edge phase ----------------
                    if l == 2:
                        ysum_ps = egy.tile([1, OC[2]], F32, tag="ysum")
                        n_mm = sum(nb for (_, _, nb, _) in pl.iters)
                        i_mm = 0
                    for (K, c0, nb, coff) in pl.iters:
                        ns = K * nb
                        g = eg.tile([P, NSMAX * RL[0]], F16, tag="g")
                        for k in range(ns):
                            nc.gpsimd.indirect_dma_start(
                                out=g[:, k * rl:(k + 1) * rl], out_offset=None,
                                in_=hcf[l][:],
                                in_offset=bass.IndirectOffsetOnAxis(
                                    ap=idx_sb[:, coff + k:coff + k + 1], axis=0))
                        gv = g[:, :ns * rl].rearrange("p (s r) -> p s r", r=rl)
                        # logits
                        lg = eg1.tile([P, NSMAX * 8], F16, tag="lg")
                        al_d_bc = alD_l[:, c0 * 8:(c0 + nb) * 8] \
                            .rearrange("p (q e) -> p q e", e=8)[:, :, :heads] \
                            .unsqueeze(2).to_broadcast([P, nb, K, heads])
                        nc.vector.tensor_tensor(
                            out=lg[:, :ns * heads].rearrange(
                                "p (q k h) -> p q k h", k=K, h=heads),
                            in0=gv[:, :, oc:oc + heads]
                            .rearrange("p (q k) h -> p q k h", k=K),
                            in1=al_d_bc, op=mybir.AluOpType.add)
                        lg2 = eg1.tile([P, NSMAX * 8], F16, tag="lg2")
                        nc.vector.tensor_scalar_mul(
                            lg2[:, :ns * heads], lg[:, :ns * heads], NEG_SLOPE)
                        lg3 = eg1.tile([P, NSMAX * 8], F16, tag="lg3")
                        nc.vector.tensor_tensor(
                            out=lg3[:, :ns * heads], in0=lg[:, :ns * heads],
                            in1=lg2[:, :ns * heads], op=mybir.AluOpType.max)
                        es = eg1.tile([P, NSMAX * 8], F16, tag="es")
                        nc.scalar.activation(
                            out=es[:, :ns * heads], in_=lg3[:, :ns * heads],
                            func=mybir.ActivationFunctionType.Exp,
                            bias=ebias_col[:])
                        # denominator
                        den = eg1.tile([P, MAXB * 8], F32, tag="den")
                        nc.vector.tensor_reduce(
                            out=den[:, :nb * heads],
                            in_=es[:, :ns * heads].rearrange(
                                "p (q k h) -> p q h k", k=K, h=heads),
                            axis=mybir.AxisListType.X, op=mybir.AluOpType.add)
                        den2 = eg1.tile([P, MAXB * 8], F32, tag="den2")
                        m_bc = mask_sb[:, c0:c0 + nb].unsqueeze(2) \
                            .to_broadcast([P, nb, heads])
                        nc.vector.tensor_tensor(
                            out=den2[:, :nb * heads].rearrange(
                                "p (q h) -> p q h", h=heads),
                            in0=den[:, :nb * heads].rearrange(
                                "p (q h) -> p q h", h=heads),
                            in1=m_bc, op=mybir.AluOpType.add)
                        inv = eg1.tile([P, MAXB * 8], F32, tag="inv")
                        nc.vector.reciprocal(out=inv[:, :nb * heads],
                                             in_=den2[:, :nb * heads])
                        # weighted messages
                        mp = eg1.tile([P, NSMAX * 128], F16, tag="mp")
                        es_bc = es[:, :ns * heads].rearrange(
                            "p (s h) -> p s h", h=heads) \
                            .unsqueeze(3).to_broadcast([P, ns, heads, ch])
                        nc.vector.tensor_tensor(
                            out=mp[:, :ns * oc].rearrange(
                                "p (s h c) -> p s h c", h=heads, c=ch),
                            in0=gv[:, :, 0:oc].rearrange(
                                "p s (h c) -> p s h c", h=heads),
                            in1=es_bc, op=mybir.AluOpType.mult)
                        # fold-tree reduce over K
                        scrA = eg1.tile([P, pl.SC1 * 128], F16, tag="scrA")
                        scrB = eg1.tile([P, pl.SC2 * 128], F16, tag="scrB")
                        cur, curk = mp, K
                        while curk > 1:
                            a_in = cur[:, :nb * curk * oc].rearrange(
                                "p (q k c) -> p q k c", k=curk, c=oc)
                            half = (curk + 1) // 2
                            pair = curk - half
                            dst_t = scrA if cur is not scrA else scrB
                            o_v = dst_t[:, :nb * half * oc].rearrange(
                                "p (q k c) -> p q k c", k=half, c=oc)
                            nc.vector.tensor_tensor(
                                out=o_v[:, :, 0:pair], in0=a_in[:, :, 0:pair],
                                in1=a_in[:, :, half:half + pair],
                                op=mybir.AluOpType.add)
                            if half > pair:
                                nc.vector.tensor_copy(
                                    out=o_v[:, :, pair:half],
                                    in_=a_in[:, :, pair:half])
                            cur, curk = dst_t, half
                        # normalize
                        hout = eg.tile([P, MAXB * 128], F16, tag="hout")
                        inv_bc = inv[:, :nb * heads].rearrange(
                            "p (q h) -> p q h", h=heads) \
                            .unsqueeze(3).to_broadcast([P, nb, heads, ch])
                        nc.vector.tensor_tensor(
                            out=hout[:, :nb * oc].rearrange(
                                "p (q h c) -> p q h c", h=heads, c=ch),
                            in0=cur[:, :nb * oc].rearrange(
                                "p (q h c) -> p q h c", h=heads, c=ch),
                            in1=inv_bc, op=mybir.AluOpType.mult)
                        if l < 2:
                            hr = eg.tile([P, MAXB * 128], F16, tag="hr")
                            nc.scalar.activation(
                                out=hr[:, :nb * oc], in_=hout[:, :nb * oc],
                                func=mybir.ActivationFunctionType.Relu)
                            for q in range(nb):
                                tp2 = egp.tile([P, P], F16, tag="tp2")
                                nc.tensor.transpose(
                                    out=tp2[:], in_=hr[:, q * oc:(q + 1) * oc],
                                    identity=ident[:])
                                nc.vector.tensor_copy(
                                    out=hout_t[:, (c0 + q) * P:(c0 + q + 1) * P],
                                    in_=tp2[:])
                        else:
                            for q in range(nb):
                                nc.tensor.matmul(
                                    ysum_ps[:], lhsT=ones_col[:],
                                    rhs=hout[:, q * oc:(q + 1) * oc],
                                    start=(i_mm == 0), stop=(i_mm == n_mm - 1))
                                i_mm += 1
                        if l < 2:
                            avail = c0 + nb
                            while (next_d < n_chunks and
                                   next_d + min(DB, n_chunks - next_d) <= avail):
                                emit_dense(l + 1, next_d)
                                next_d += DB
                nc.vector.tensor_copy(out=ysb[:], in_=ysum_ps[:])
            nc.sync.dma_start(t_y.ap(), ysb[:])
    nc.compile()
    return nc


# ----------------------------------------------------------------- entry point

_CACHE = {}


def _get_compiled(edge_index, reps=1):
    key = (hash(edge_index.tobytes()), reps)
    if key not in _CACHE:
        pl = _CACHE.get(("plan", hash(edge_index.tobytes())))
        if pl is None:
            pl = make_plan(edge_index, N_NODES, ncores=8)
            _CACHE[("plan", hash(edge_index.tobytes()))] = pl
        _CACHE[key] = (pl, build_program(pl, reps=reps))
    return _CACHE[key]


def make_inputs(pl, x, edge_index, W1, a_src1, a_dst1, b1, W2, a_src2, a_dst2,
                b2, W3, a_src3, a_dst3, b3):
    hins = x_slices(pl, np.asarray(x, np.float32))
    Ws = [np.asarray(W, np.float16) for W in (W1, W2, W3)]
    avs = [pack_av(a_src1, a_dst1, OC[0]), pack_av(a_src2, a_dst2, OC[1]),
           pack_av(a_src3, a_dst3, OC[2])]
    bvs = [np.tile(np.asarray(b, np.float16)[None, :], (P, 1)) for b in (b1, b2)]
    in_maps = []
    for c in range(pl.ncores):
        in_maps.append({
            "hin": hins[c], "idx": pl.idx[c], "maskD": pl.maskD[c],
            "W1": Ws[0], "W2": Ws[1], "W3": Ws[2],
            "av1": avs[0], "av2": avs[1], "av3": avs[2],
            "bv1": bvs[0], "bv2": bvs[1],
        })
    return in_maps


def kernel(x, edge_index, W1, a_src1, a_dst1, b1, W2, a_src2, a_dst2, b2,
           W3, a_src3, a_dst3, b3):
    from concourse import bass_utils
    edge_index = np.asarray(edge_index, np.int32)
    pl, nc = _get_compiled(edge_index)
    in_maps = make_inputs(pl, x, edge_index, W1, a_src1, a_dst1, b1,
                          W2, a_src2, a_dst2, b2, W3, a_src3, a_dst3, b3)
    res = bass_utils.run_bass_kernel_spmd(nc, in_maps, core_ids=list(range(8)))
    tot = np.sum([res.results[c]["y"] for c in range(8)], axis=0)
    return (tot / np.float32(N_NODES)
            + np.asarray(b3, np.float32)[None, :]).astype(np.float32)
